# revision 17
# baseline (speedup 1.0000x reference)
"""Bass/Trainium2 kernel for ExtendedTripletLoss (data-parallel over batch).

Math: for each pair (f1,m1),(f2,m2) and shift off in [-4,4]:
  num(off) = sum mask*(f1-f2r)^2 = t1 + t2 - 2*t3
    t1 = corr(A, m2)(off),   A  = sum_c (m1*f1)^2        [32,512]
    t2 = corr(m1, B2)(off),  B2 = sum_c (m2*f2)^2        [32,512]
    t3 = corr(U, V)(off),    U = m1*f1, V = m2*f2        [512,512]
  den(off) = C * corr(m1, m2)(off) + 1e-3
All correlations at 9 lags are computed on TensorE as Gram-block matmuls:
contraction over rows (c,h), w blocked 4x128; rhs uses a +-4 padded copy so
each block's 136-wide window holds all 9 shifted columns. All 4 w-blocks and
all terms accumulate into ONE PSUM tile [128,136]; lag sums are the 9
diagonals col = i + 4 - off, extracted on the host from the DMA'd blocks.
"""

import os
import sys
from contextlib import ExitStack

import numpy as np

for _p in ("/opt/trn_rl_repo", "/root/.axon_site/_ro/trn_rl_repo"):
    if os.path.isdir(_p) and _p not in sys.path:
        sys.path.insert(0, _p)
        break

import ml_dtypes

import concourse.bass as bass
import concourse.mybir as mybir
import concourse.tile as tile
# This environment's walrus_driver allows only ONE sync-wait per instruction,
# while Tile freely aggregates several. Post-pass: move excess waits onto
# freshly inserted same-engine NOPs directly before the instruction.
_MAXW = 1


def _split_waits_pass(nc):
    n = 0
    for fn in nc.m.functions:
        for blk in fn.blocks:
            out = []
            changed = False
            for inst in blk.instructions:
                si = inst.sync_info
                waits = list(si.on_wait) if si is not None else []
                if len(waits) > _MAXW:
                    for i in range(0, len(waits) - _MAXW, _MAXW):
                        nop = mybir.InstNoOp(name=f"{inst.name}-wsplit{i}")
                        nop.engine = inst.engine
                        nop.sync_info = mybir.SyncInfo(
                            on_update=[], on_wait=waits[i : i + _MAXW]
                        )
                        out.append(nop)
                        n += 1
                    si.on_wait = waits[len(waits) - _MAXW :]
                    changed = True
                out.append(inst)
            if changed:
                blk.instructions = out
    return n


BF16 = mybir.dt.bfloat16
F32 = mybir.dt.float32

B, C, H, W = 64, 16, 32, 512
NCORES = 8
S = B // NCORES          # samples per core
R = C * H                # 512 rows in (c,h) contraction dim
NB = R // 128            # 4 partition chunks
JB = W // 128            # 4 w-blocks
NW = 136                 # window width = 128 + 2*4
MARGIN = 0.15
SHIFT = 4

_nc_cache = None


def build_nc(for_hw=True):
    nc = bass.Bass()
    x_a = nc.declare_dram_parameter("x_a", [S, R, W], BF16, isOutput=False)
    x_p = nc.declare_dram_parameter("x_p", [S, R, W], BF16, isOutput=False)
    x_n = nc.declare_dram_parameter("x_n", [S, R, W], BF16, isOutput=False)
    masks = nc.declare_dram_parameter("masks", [S, 3 * H, W], BF16, isOutput=False)
    ind = nc.declare_dram_parameter("ind", [128, H], BF16, isOutput=False)
    # raw[s, pair, i, kind, c]: kind 0 = num-gram, kind 1 = den-gram
    raw = nc.declare_dram_parameter("raw", [S, 2, 128, 2, NW], F32, isOutput=True)

    with tile.TileContext(nc) as tc, ExitStack() as ctx:
        const = ctx.enter_context(tc.tile_pool(name="const", bufs=1))
        io = ctx.enter_context(tc.tile_pool(name="io", bufs=2))
        mk = ctx.enter_context(tc.tile_pool(name="mk", bufs=2))
        um = ctx.enter_context(tc.tile_pool(name="um", bufs=2))
        sq = ctx.enter_context(tc.tile_pool(name="sq", bufs=2))
        k4p = ctx.enter_context(tc.tile_pool(name="k4p", bufs=2))
        outsb = ctx.enter_context(tc.tile_pool(name="outsb", bufs=4))
        indps = ctx.enter_context(tc.tile_pool(name="indps", bufs=3, space="PSUM"))
        gram = ctx.enter_context(tc.tile_pool(name="gram", bufs=2, space="PSUM"))

        ind_sb = const.tile([128, H], BF16)
        nc.sync.dma_start(out=ind_sb, in_=ind[:])

        mult = mybir.AluOpType.mult

        for s in range(S):
            # ---- loads ----
            abuf = io.tile([128, NB, W], BF16, tag="abuf")
            pbuf = io.tile([128, NB, W], BF16, tag="pbuf")
            nbuf = io.tile([128, NB, W], BF16, tag="nbuf")
            nc.sync.dma_start(out=abuf, in_=x_a[s].rearrange("(j p) w -> p j w", p=128))
            nc.sync.dma_start(out=pbuf, in_=x_p[s].rearrange("(j p) w -> p j w", p=128))
            nc.sync.dma_start(out=nbuf, in_=x_n[s].rearrange("(j p) w -> p j w", p=128))
            mallb = mk.tile([3 * H, W], BF16, tag="mallb")
            nc.sync.dma_start(out=mallb, in_=masks[s])

            # ---- mask replicas to 128 partitions (4 vertical copies each) ----
            mrep = mk.tile([128, 3, W], BF16, tag="mrep")
            for t in range(3):
                for rep in range(NB):
                    nc.sync.dma_start(
                        out=mrep[32 * rep : 32 * rep + 32, t, :],
                        in_=mallb[32 * t : 32 * t + 32, :],
                    )

            def rep_b(t):
                # [128, NB, W] view of mrep[:, t, :] broadcast over the NB axis
                return mrep[:, t, :].unsqueeze(1).broadcast_to((128, NB, W))

            # ---- masking (DVE): U = ma*a ; Vx = -2*mp*p ; Wx = -2*mn*n ----
            ubuf = um.tile([128, NB, W], BF16, tag="ubuf")
            vext = um.tile([128, NB, W + 8], BF16, tag="vext")
            wext = um.tile([128, NB, W + 8], BF16, tag="wext")
            nc.vector.tensor_tensor(out=ubuf, in0=abuf, in1=rep_b(0), op=mult)
            nc.vector.scalar_tensor_tensor(
                out=vext[:, :, 4 : W + 4], in0=pbuf, scalar=-2.0, in1=rep_b(1),
                op0=mult, op1=mult,
            )
            nc.vector.scalar_tensor_tensor(
                out=wext[:, :, 4 : W + 4], in0=nbuf, scalar=-2.0, in1=rep_b(2),
                op0=mult, op1=mult,
            )
            # circular wrap columns
            for ext in (vext, wext):
                nc.vector.tensor_copy(out=ext[:, :, 0:4], in_=ext[:, :, W : W + 4])
                nc.vector.tensor_copy(out=ext[:, :, W + 4 : W + 8], in_=ext[:, :, 4:8])

            # ---- squares (ACT) ----
            u2 = sq.tile([128, NB, W], BF16, tag="u2")
            v2 = sq.tile([128, NB, W], BF16, tag="v2")
            w2 = sq.tile([128, NB, W], BF16, tag="w2")
            Sq = mybir.ActivationFunctionType.Square
            nc.scalar.activation(out=u2, in_=ubuf, func=Sq)
            nc.scalar.activation(out=v2, in_=vext[:, :, 4 : W + 4], func=Sq, scale=0.5)
            nc.scalar.activation(out=w2, in_=wext[:, :, 4 : W + 4], func=Sq, scale=0.5)

            # ---- c-reduction via indicator matmul: A/Bp/Bn [32, 512] ----
            a_ps = indps.tile([H, W], F32, tag="ind3")
            b_ps = indps.tile([H, W], F32, tag="ind3")
            c_ps = indps.tile([H, W], F32, tag="ind3")
            for j in range(NB):
                nc.tensor.matmul(a_ps, ind_sb, u2[:, j, :], start=(j == 0), stop=(j == NB - 1))
            for j in range(NB):
                nc.tensor.matmul(b_ps, ind_sb, v2[:, j, :], start=(j == 0), stop=(j == NB - 1))
            for j in range(NB):
                nc.tensor.matmul(c_ps, ind_sb, w2[:, j, :], start=(j == 0), stop=(j == NB - 1))

            # ---- assemble k4 lhsT [A; ma] and rhs [m2_ext; B2_ext] ----
            Cp = mybir.ActivationFunctionType.Copy
            k4lhs = k4p.tile([2 * H, W], BF16, tag="k4lhs")
            rhs4p = k4p.tile([2 * H, W + 8], BF16, tag="rhs4p")
            rhs4n = k4p.tile([2 * H, W + 8], BF16, tag="rhs4n")
            nc.scalar.activation(out=k4lhs[0:H, :], in_=a_ps, func=Cp)
            nc.vector.tensor_copy(out=k4lhs[H : 2 * H, :], in_=mallb[0:H, :])
            nc.vector.tensor_copy(out=rhs4p[0:H, 4 : W + 4], in_=mallb[H : 2 * H, :])
            nc.scalar.activation(out=rhs4p[H : 2 * H, 4 : W + 4], in_=b_ps, func=Cp)
            nc.vector.tensor_copy(out=rhs4n[0:H, 4 : W + 4], in_=mallb[2 * H : 3 * H, :])
            nc.scalar.activation(out=rhs4n[H : 2 * H, 4 : W + 4], in_=c_ps, func=Cp)
            for r4 in (rhs4p, rhs4n):
                nc.vector.tensor_copy(out=r4[:, 0:4], in_=r4[:, W : W + 4])
                nc.vector.tensor_copy(out=r4[:, W + 4 : W + 8], in_=r4[:, 4:8])

            # ---- Gram matmuls ----
            for pair, (ext, r4) in enumerate(((vext, rhs4p), (wext, rhs4n))):
                num_ps = gram.tile([128, NW], F32, tag="num")
                den_ps = gram.tile([128, NW], F32, tag="den")
                for j1 in range(JB):
                    mb = slice(j1 * 128, (j1 + 1) * 128)
                    wn = slice(j1 * 128, j1 * 128 + NW)
                    for kc in range(NB):
                        nc.tensor.matmul(
                            num_ps, ubuf[:, kc, mb], ext[:, kc, wn],
                            start=(j1 == 0 and kc == 0), stop=False,
                        )
                    nc.tensor.matmul(
                        num_ps, k4lhs[:, mb], r4[:, wn],
                        start=False, stop=(j1 == JB - 1),
                    )
                    nc.tensor.matmul(
                        den_ps, mallb[0:H, mb], r4[0:H, wn],
                        start=(j1 == 0), stop=(j1 == JB - 1),
                    )
                psb = outsb.tile([128, 2, NW], F32, tag="psb")
                nc.scalar.activation(out=psb[:, 0, :], in_=num_ps, func=Cp)
                nc.scalar.activation(out=psb[:, 1, :], in_=den_ps, func=Cp)
                nc.sync.dma_start(out=raw[s, pair], in_=psb)
    if for_hw:
        _split_waits_pass(nc)
    return nc


def _host_prep(a, p, n, ma, mp, mn):
    bf = ml_dtypes.bfloat16
    A = np.ascontiguousarray(a.reshape(B, R, W)).astype(bf)
    P = np.ascontiguousarray(p.reshape(B, R, W)).astype(bf)
    N = np.ascontiguousarray(n.reshape(B, R, W)).astype(bf)
    M = np.concatenate(
        [ma.reshape(B, H, W), mp.reshape(B, H, W), mn.reshape(B, H, W)], axis=1
    ).astype(bf)
    ind = np.zeros((128, H), dtype=bf)
    ind[np.arange(128), np.arange(128) % H] = 1
    in_maps = []
    for c in range(NCORES):
        sl = slice(c * S, (c + 1) * S)
        in_maps.append(
            {"x_a": A[sl], "x_p": P[sl], "x_n": N[sl], "masks": M[sl], "ind": ind}
        )
    return in_maps


def _host_finish(raw_all):
    # raw_all: [B, 2, 128, 2, NW] float32
    raw64 = raw_all.astype(np.float64)
    nums = raw64[:, :, :, 0]  # [B, 2, 128, NW]
    dens = raw64[:, :, :, 1]
    idx = np.arange(128)
    dists = []
    for off in range(-SHIFT, SHIFT + 1):
        cols = idx + 4 - off
        num = nums[:, :, idx, cols].sum(axis=-1)   # [B, 2]
        cnt = dens[:, :, idx, cols].sum(axis=-1)
        dists.append(num / (C * cnt + 0.001))
    d = np.min(np.stack(dists, axis=0), axis=0)    # [B, 2]
    loss = np.maximum(d[:, 0] - d[:, 1] + MARGIN, 0.0)
    return np.array(loss.mean(), dtype=np.float32)


def kernel(a, p, n, ma, mp, mn):
    global _nc_cache
    from concourse import bass_utils

    if _nc_cache is None:
        _nc_cache = build_nc()
    nc = _nc_cache
    in_maps = _host_prep(a, p, n, ma, mp, mn)
    res = bass_utils.run_bass_kernel_spmd(nc, in_maps, core_ids=list(range(NCORES)))
    raw_all = np.concatenate([res.results[i]["raw"] for i in range(NCORES)], axis=0)
    return _host_finish(raw_all)


# revision 26
# speedup vs baseline: 1.1792x; 1.1792x over previous
"""Bass/Trainium2 kernel for ExtendedTripletLoss (data-parallel over batch).

Math: for each pair (f1,m1),(f2,m2) and shift off in [-4,4]:
  num(off) = sum mask*(f1-f2r)^2 = t1 + t2 - 2*t3
    t1 = corr(A, m2)(off),   A  = sum_c (m1*f1)^2        [32,512]
    t2 = corr(m1, B2)(off),  B2 = sum_c (m2*f2)^2        [32,512]
    t3 = corr(U, V)(off),    U = m1*f1, V = m2*f2        [512,512]
  den(off) = C * corr(m1, m2)(off) + 1e-3
All correlations at 9 lags are computed on TensorE as Gram-block matmuls:
contraction over rows (c,h), w blocked 4x128; rhs uses a +-4 padded copy so
each block's 136-wide window holds all 9 shifted columns. All 4 w-blocks and
all terms accumulate into ONE PSUM tile [128,136]; lag sums are the 9
diagonals col = i + 4 - off, extracted on the host from the DMA'd blocks.
"""

import os
import sys
from contextlib import ExitStack

import numpy as np

for _p in ("/opt/trn_rl_repo", "/root/.axon_site/_ro/trn_rl_repo"):
    if os.path.isdir(_p) and _p not in sys.path:
        sys.path.insert(0, _p)
        break

import ml_dtypes

import concourse.bass as bass
import concourse.mybir as mybir
import concourse.tile as tile
# This environment's walrus_driver allows only ONE sync-wait per instruction,
# while Tile freely aggregates several. Post-pass: move excess waits onto
# freshly inserted same-engine NOPs directly before the instruction.
_MAXW = 1


def _split_waits_pass(nc):
    n = 0
    for fn in nc.m.functions:
        for blk in fn.blocks:
            out = []
            changed = False
            for inst in blk.instructions:
                si = inst.sync_info
                waits = list(si.on_wait) if si is not None else []
                if len(waits) > _MAXW:
                    for i in range(0, len(waits) - _MAXW, _MAXW):
                        nop = mybir.InstNoOp(name=f"{inst.name}-wsplit{i}")
                        nop.engine = inst.engine
                        nop.sync_info = mybir.SyncInfo(
                            on_update=[], on_wait=waits[i : i + _MAXW]
                        )
                        out.append(nop)
                        n += 1
                    si.on_wait = waits[len(waits) - _MAXW :]
                    changed = True
                out.append(inst)
            if changed:
                blk.instructions = out
    return n


BF16 = mybir.dt.bfloat16
F32 = mybir.dt.float32

B, C, H, W = 64, 16, 32, 512
NCORES = 8
S = B // NCORES          # samples per core
R = C * H                # 512 rows in (c,h) contraction dim
NB = R // 128            # 4 partition chunks
JB = W // 128            # 4 w-blocks
NW = 136                 # window width = 128 + 2*4
MARGIN = 0.15
SHIFT = 4

_nc_cache = None


def build_nc(for_hw=True):
    nc = bass.Bass()
    x_a = nc.declare_dram_parameter("x_a", [S, R, W], BF16, isOutput=False)
    x_p = nc.declare_dram_parameter("x_p", [S, R, W], BF16, isOutput=False)
    x_n = nc.declare_dram_parameter("x_n", [S, R, W], BF16, isOutput=False)
    # masks_ext: circularly padded along W: [:, 0:4]=m[:, 508:512],
    # [:, 4:516]=m, [:, 516:520]=m[:, 0:4]; rows = [ma; mp; mn]
    masks = nc.declare_dram_parameter("masks", [S, 3 * H, W + 8], BF16, isOutput=False)
    ind = nc.declare_dram_parameter("ind", [128, H], BF16, isOutput=False)
    # raw[s, pair, i, kind, c]: kind 0 = num-gram, kind 1 = den-gram
    raw = nc.declare_dram_parameter("raw", [S, 2, 128, 2, NW], F32, isOutput=True)

    with tile.TileContext(nc) as tc, ExitStack() as ctx:
        const = ctx.enter_context(tc.tile_pool(name="const", bufs=1))
        io = ctx.enter_context(tc.tile_pool(name="io", bufs=3))
        mk = ctx.enter_context(tc.tile_pool(name="mk", bufs=3))
        um = ctx.enter_context(tc.tile_pool(name="um", bufs=2))
        sq = ctx.enter_context(tc.tile_pool(name="sq", bufs=2))
        k4p = ctx.enter_context(tc.tile_pool(name="k4p", bufs=2))
        outsb = ctx.enter_context(tc.tile_pool(name="outsb", bufs=4))
        indps = ctx.enter_context(tc.tile_pool(name="indps", bufs=3, space="PSUM"))
        gram = ctx.enter_context(tc.tile_pool(name="gram", bufs=2, space="PSUM"))

        ind_sb = const.tile([128, H], BF16)
        nc.sync.dma_start(out=ind_sb, in_=ind[:])

        mult = mybir.AluOpType.mult

        for s in range(S):
            # ---- loads ----
            abuf = io.tile([128, NB, W], BF16, tag="abuf")
            pbuf = io.tile([128, NB, W], BF16, tag="pbuf")
            nbuf = io.tile([128, NB, W], BF16, tag="nbuf")
            nc.sync.dma_start(out=abuf, in_=x_a[s].rearrange("(j p) w -> p j w", p=128))
            nc.sync.dma_start(out=pbuf, in_=x_p[s].rearrange("(j p) w -> p j w", p=128))
            nc.sync.dma_start(out=nbuf, in_=x_n[s].rearrange("(j p) w -> p j w", p=128))

            # ---- mask replicas to 128 partitions: one broadcast DMA each ----
            mrep = mk.tile([128, 3, W], BF16, tag="mrep")
            for t in range(3):
                for rep in range(NB):
                    nc.gpsimd.dma_start(
                        out=mrep[H * rep : H * rep + H, t, :],
                        in_=masks[s, 32 * t : 32 * t + 32, 4 : W + 4],
                    )
            # fold the -2 of the cross-term into the mp/mn replicas
            nc.vector.tensor_scalar_mul(mrep[:, 1:3, :], mrep[:, 1:3, :], -2.0)

            def rep_b(t):
                # [128, NB, W] view of mrep[:, t, :] broadcast over the NB axis
                return mrep[:, t, :].unsqueeze(1).broadcast_to((128, NB, W))

            # ---- masking (DVE): U = ma*a ; Vx = -2*mp*p ; Wx = -2*mn*n ----
            ubuf = um.tile([128, NB, W], BF16, tag="ubuf")
            vext = um.tile([128, NB, W + 8], BF16, tag="vext")
            wext = um.tile([128, NB, W + 8], BF16, tag="wext")
            nc.vector.tensor_tensor(out=ubuf, in0=abuf, in1=rep_b(0), op=mult)
            nc.vector.tensor_tensor(
                out=vext[:, :, 4 : W + 4], in0=pbuf, in1=rep_b(1), op=mult
            )
            nc.vector.tensor_tensor(
                out=wext[:, :, 4 : W + 4], in0=nbuf, in1=rep_b(2), op=mult
            )
            # circular wrap columns
            for ext in (vext, wext):
                nc.vector.tensor_copy(out=ext[:, :, 0:4], in_=ext[:, :, W : W + 4])
                nc.vector.tensor_copy(out=ext[:, :, W + 4 : W + 8], in_=ext[:, :, 4:8])

            # ---- squares (ACT) ----
            u2 = sq.tile([128, NB, W], BF16, tag="u2")
            v2 = sq.tile([128, NB, W], BF16, tag="v2")
            w2 = sq.tile([128, NB, W], BF16, tag="w2")
            Sq = mybir.ActivationFunctionType.Square
            nc.scalar.activation(out=u2, in_=ubuf, func=Sq)
            nc.scalar.activation(out=v2, in_=vext[:, :, 4 : W + 4], func=Sq, scale=0.5)
            nc.scalar.activation(out=w2, in_=wext[:, :, 4 : W + 4], func=Sq, scale=0.5)

            # ---- c-reduction via indicator matmul: A/Bp/Bn [32, 512] ----
            a_ps = indps.tile([H, W], F32, tag="ind3")
            b_ps = indps.tile([H, W], F32, tag="ind3")
            c_ps = indps.tile([H, W], F32, tag="ind3")
            for j in range(NB):
                nc.tensor.matmul(a_ps, ind_sb, u2[:, j, :], start=(j == 0), stop=(j == NB - 1))
            for j in range(NB):
                nc.tensor.matmul(b_ps, ind_sb, v2[:, j, :], start=(j == 0), stop=(j == NB - 1))
            for j in range(NB):
                nc.tensor.matmul(c_ps, ind_sb, w2[:, j, :], start=(j == 0), stop=(j == NB - 1))

            # ---- assemble k4 lhsT [A; ma] and rhs [m2_ext; B2_ext] ----
            Cp = mybir.ActivationFunctionType.Copy
            k4lhs = k4p.tile([2 * H, W], BF16, tag="k4lhs")
            rhs4p = k4p.tile([2 * H, W + 8], BF16, tag="rhs4p")
            rhs4n = k4p.tile([2 * H, W + 8], BF16, tag="rhs4n")
            nc.scalar.activation(out=k4lhs[0:H, :], in_=a_ps, func=Cp)
            nc.gpsimd.dma_start(
                out=k4lhs[H : 2 * H, :], in_=masks[s, 0:H, 4 : W + 4]
            )
            mab = mk.tile([H, W], BF16, tag="mab")
            nc.gpsimd.dma_start(out=mab, in_=masks[s, 0:H, 4 : W + 4])
            nc.gpsimd.dma_start(out=rhs4p[0:H, :], in_=masks[s, H : 2 * H, :])
            nc.scalar.activation(out=rhs4p[H : 2 * H, 4 : W + 4], in_=b_ps, func=Cp)
            nc.gpsimd.dma_start(out=rhs4n[0:H, :], in_=masks[s, 2 * H : 3 * H, :])
            nc.scalar.activation(out=rhs4n[H : 2 * H, 4 : W + 4], in_=c_ps, func=Cp)
            for r4 in (rhs4p, rhs4n):
                nc.vector.tensor_copy(
                    out=r4[H : 2 * H, 0:4], in_=r4[H : 2 * H, W : W + 4]
                )
                nc.vector.tensor_copy(
                    out=r4[H : 2 * H, W + 4 : W + 8], in_=r4[H : 2 * H, 4:8]
                )

            # ---- Gram matmuls ----
            for pair, (ext, r4) in enumerate(((vext, rhs4p), (wext, rhs4n))):
                num_ps = gram.tile([128, NW], F32, tag="num")
                den_ps = gram.tile([128, NW], F32, tag="den")
                for j1 in range(JB):
                    mb = slice(j1 * 128, (j1 + 1) * 128)
                    wn = slice(j1 * 128, j1 * 128 + NW)
                    for kc in range(NB):
                        nc.tensor.matmul(
                            num_ps, ubuf[:, kc, mb], ext[:, kc, wn],
                            start=(j1 == 0 and kc == 0), stop=False,
                        )
                    nc.tensor.matmul(
                        num_ps, k4lhs[:, mb], r4[:, wn],
                        start=False, stop=(j1 == JB - 1),
                    )
                    nc.tensor.matmul(
                        den_ps, mab[:, mb], r4[0:H, wn],
                        start=(j1 == 0), stop=(j1 == JB - 1),
                    )
                psb = outsb.tile([128, 2, NW], F32, tag="psb")
                nc.scalar.activation(out=psb[:, 0, :], in_=num_ps, func=Cp)
                nc.vector.tensor_copy(out=psb[:, 1, :], in_=den_ps)
                nc.gpsimd.dma_start(out=raw[s, pair], in_=psb)
    if for_hw:
        _split_waits_pass(nc)
    return nc


def _host_prep(a, p, n, ma, mp, mn):
    bf = ml_dtypes.bfloat16
    A = np.ascontiguousarray(a.reshape(B, R, W)).astype(bf)
    P = np.ascontiguousarray(p.reshape(B, R, W)).astype(bf)
    N = np.ascontiguousarray(n.reshape(B, R, W)).astype(bf)
    M0 = np.concatenate(
        [ma.reshape(B, H, W), mp.reshape(B, H, W), mn.reshape(B, H, W)], axis=1
    ).astype(bf)
    M = np.concatenate([M0[:, :, W - 4 :], M0, M0[:, :, :4]], axis=2)
    ind = np.zeros((128, H), dtype=bf)
    ind[np.arange(128), np.arange(128) % H] = 1
    in_maps = []
    for c in range(NCORES):
        sl = slice(c * S, (c + 1) * S)
        in_maps.append(
            {"x_a": A[sl], "x_p": P[sl], "x_n": N[sl], "masks": M[sl], "ind": ind}
        )
    return in_maps


def _host_finish(raw_all):
    # raw_all: [B, 2, 128, 2, NW] float32
    raw64 = raw_all.astype(np.float64)
    nums = raw64[:, :, :, 0]  # [B, 2, 128, NW]
    dens = raw64[:, :, :, 1]
    idx = np.arange(128)
    dists = []
    for off in range(-SHIFT, SHIFT + 1):
        cols = idx + 4 - off
        num = nums[:, :, idx, cols].sum(axis=-1)   # [B, 2]
        cnt = dens[:, :, idx, cols].sum(axis=-1)
        dists.append(num / (C * cnt + 0.001))
    d = np.min(np.stack(dists, axis=0), axis=0)    # [B, 2]
    loss = np.maximum(d[:, 0] - d[:, 1] + MARGIN, 0.0)
    return np.array(loss.mean(), dtype=np.float32)


def kernel(a, p, n, ma, mp, mn):
    global _nc_cache
    from concourse import bass_utils

    if _nc_cache is None:
        _nc_cache = build_nc()
    nc = _nc_cache
    in_maps = _host_prep(a, p, n, ma, mp, mn)
    res = bass_utils.run_bass_kernel_spmd(nc, in_maps, core_ids=list(range(NCORES)))
    raw_all = np.concatenate([res.results[i]["raw"] for i in range(NCORES)], axis=0)
    return _host_finish(raw_all)


# revision 32
# speedup vs baseline: 1.5367x; 1.3032x over previous
"""Bass/Trainium2 kernel for ExtendedTripletLoss (data-parallel over batch).

Math: for each pair (f1,m1),(f2,m2) and shift off in [-4,4]:
  num(off) = sum mask*(f1-f2r)^2 = t1 + t2 - 2*t3
    t1 = corr(A, m2)(off),   A  = sum_c (m1*f1)^2        [32,512]
    t2 = corr(m1, B2)(off),  B2 = sum_c (m2*f2)^2        [32,512]
    t3 = corr(U, V)(off),    U = m1*f1, V = m2*f2        [512,512]
  den(off) = C * corr(m1, m2)(off) + 1e-3
All correlations at 9 lags are computed on TensorE as Gram-block matmuls:
contraction over rows (c,h), w blocked 4x128; rhs uses a +-4 padded copy so
each block's 136-wide window holds all 9 shifted columns. All 4 w-blocks and
all terms accumulate into ONE PSUM tile [128,136]; lag sums are the 9
diagonals col = i + 4 - off, extracted on the host from the DMA'd blocks.
"""

import os
import sys
from contextlib import ExitStack

import numpy as np

for _p in ("/opt/trn_rl_repo", "/root/.axon_site/_ro/trn_rl_repo"):
    if os.path.isdir(_p) and _p not in sys.path:
        sys.path.insert(0, _p)
        break

import ml_dtypes

import concourse.bass as bass
import concourse.mybir as mybir
import concourse.tile as tile
# This environment's walrus_driver allows only ONE sync-wait per instruction,
# while Tile freely aggregates several. Post-pass: move excess waits onto
# freshly inserted same-engine NOPs directly before the instruction.
_MAXW = 1


def _split_waits_pass(nc):
    n = 0
    for fn in nc.m.functions:
        for blk in fn.blocks:
            out = []
            changed = False
            for inst in blk.instructions:
                si = inst.sync_info
                waits = list(si.on_wait) if si is not None else []
                if len(waits) > _MAXW:
                    for i in range(0, len(waits) - _MAXW, _MAXW):
                        nop = mybir.InstNoOp(name=f"{inst.name}-wsplit{i}")
                        nop.engine = inst.engine
                        nop.sync_info = mybir.SyncInfo(
                            on_update=[], on_wait=waits[i : i + _MAXW]
                        )
                        out.append(nop)
                        n += 1
                    si.on_wait = waits[len(waits) - _MAXW :]
                    changed = True
                out.append(inst)
            if changed:
                blk.instructions = out
    return n


BF16 = mybir.dt.bfloat16
F32 = mybir.dt.float32

B, C, H, W = 64, 16, 32, 512
NCORES = 8
S = B // NCORES          # samples per core
R = C * H                # 512 rows in (c,h) contraction dim
NB = R // 128            # 4 partition chunks
JB = W // 128            # 4 w-blocks
NW = 136                 # window width = 128 + 2*4
MARGIN = 0.15
SHIFT = 4

_nc_cache = None


def build_nc(for_hw=True):
    nc = bass.Bass()
    x_a = nc.declare_dram_parameter("x_a", [S, R, W], BF16, isOutput=False)
    x_p = nc.declare_dram_parameter("x_p", [S, R, W], BF16, isOutput=False)
    x_n = nc.declare_dram_parameter("x_n", [S, R, W], BF16, isOutput=False)
    # masks_ext: circularly padded along W: [:, 0:4]=m[:, 508:512],
    # [:, 4:516]=m, [:, 516:520]=m[:, 0:4]; rows = [ma; mp; mn]
    masks = nc.declare_dram_parameter("masks", [S, 3 * H, W + 8], BF16, isOutput=False)
    # mask replicas: [s, p, t, w] = mask_t[p % 32, w], with t=1,2 pre-scaled by -2
    masks_rep = nc.declare_dram_parameter("masks_rep", [S, 128, 3, W], BF16, isOutput=False)
    ind = nc.declare_dram_parameter("ind", [128, H], BF16, isOutput=False)
    # raw[s, pair, i, kind, c]: kind 0 = num-gram, kind 1 = den-gram
    raw = nc.declare_dram_parameter("raw", [S, 2, 128, 2, NW], F32, isOutput=True)

    with tile.TileContext(nc) as tc, ExitStack() as ctx:
        const = ctx.enter_context(tc.tile_pool(name="const", bufs=1))
        io = ctx.enter_context(tc.tile_pool(name="io", bufs=3))
        mk = ctx.enter_context(tc.tile_pool(name="mk", bufs=3))
        um = ctx.enter_context(tc.tile_pool(name="um", bufs=2))
        sq = ctx.enter_context(tc.tile_pool(name="sq", bufs=2))
        k4p = ctx.enter_context(tc.tile_pool(name="k4p", bufs=2))
        outsb = ctx.enter_context(tc.tile_pool(name="outsb", bufs=4))
        indps = ctx.enter_context(tc.tile_pool(name="indps", bufs=3, space="PSUM"))
        gram = ctx.enter_context(tc.tile_pool(name="gram", bufs=2, space="PSUM"))

        ind_sb = const.tile([128, H], BF16)
        nc.sync.dma_start(out=ind_sb, in_=ind[:])

        mult = mybir.AluOpType.mult

        for s in range(S):
            # ---- loads ----
            abuf = io.tile([128, NB, W], BF16, tag="abuf")
            pbuf = io.tile([128, NB, W], BF16, tag="pbuf")
            nbuf = io.tile([128, NB, W], BF16, tag="nbuf")
            nc.sync.dma_start(out=abuf, in_=x_a[s].rearrange("(j p) w -> p j w", p=128))
            nc.sync.dma_start(out=pbuf, in_=x_p[s].rearrange("(j p) w -> p j w", p=128))
            nc.sync.dma_start(out=nbuf, in_=x_n[s].rearrange("(j p) w -> p j w", p=128))

            # ---- mask replicas to 128 partitions: one broadcast DMA each ----
            mrep = mk.tile([128, 3, W], BF16, tag="mrep")
            nc.gpsimd.dma_start(out=mrep, in_=masks_rep[s])

            def rep_b(t):
                # [128, NB, W] view of mrep[:, t, :] broadcast over the NB axis
                return mrep[:, t, :].unsqueeze(1).broadcast_to((128, NB, W))

            # ---- masking (DVE): U = ma*a ; Vx = -2*mp*p ; Wx = -2*mn*n ----
            ubuf = um.tile([128, NB, W], BF16, tag="ubuf")
            vext = um.tile([128, NB, W + 8], BF16, tag="vext")
            wext = um.tile([128, NB, W + 8], BF16, tag="wext")
            nc.vector.tensor_tensor(out=ubuf, in0=abuf, in1=rep_b(0), op=mult)
            nc.vector.tensor_tensor(
                out=vext[:, :, 4 : W + 4], in0=pbuf, in1=rep_b(1), op=mult
            )
            nc.vector.tensor_tensor(
                out=wext[:, :, 4 : W + 4], in0=nbuf, in1=rep_b(2), op=mult
            )
            # circular wrap columns
            for ext in (vext, wext):
                nc.vector.tensor_copy(out=ext[:, :, 0:4], in_=ext[:, :, W : W + 4])
                nc.vector.tensor_copy(out=ext[:, :, W + 4 : W + 8], in_=ext[:, :, 4:8])

            # ---- squares (ACT) ----
            u2 = sq.tile([128, NB, W], BF16, tag="u2")
            v2 = sq.tile([128, NB, W], BF16, tag="v2")
            w2 = sq.tile([128, NB, W], BF16, tag="w2")
            Sq = mybir.ActivationFunctionType.Square
            nc.scalar.activation(out=u2, in_=ubuf, func=Sq)
            nc.scalar.activation(out=v2, in_=vext[:, :, 4 : W + 4], func=Sq, scale=0.5)
            nc.scalar.activation(out=w2, in_=wext[:, :, 4 : W + 4], func=Sq, scale=0.5)

            # ---- c-reduction via indicator matmul: A/Bp/Bn [32, 512] ----
            a_ps = indps.tile([H, W], F32, tag="ind3")
            b_ps = indps.tile([H, W], F32, tag="ind3")
            c_ps = indps.tile([H, W], F32, tag="ind3")
            for j in range(NB):
                nc.tensor.matmul(a_ps, ind_sb, u2[:, j, :], start=(j == 0), stop=(j == NB - 1))
            for j in range(NB):
                nc.tensor.matmul(b_ps, ind_sb, v2[:, j, :], start=(j == 0), stop=(j == NB - 1))
            for j in range(NB):
                nc.tensor.matmul(c_ps, ind_sb, w2[:, j, :], start=(j == 0), stop=(j == NB - 1))

            # ---- assemble k4 lhsT [A; ma] and rhs [m2_ext; B2_ext] ----
            Cp = mybir.ActivationFunctionType.Copy
            k4lhs = k4p.tile([2 * H, W], BF16, tag="k4lhs")
            rhs4p = k4p.tile([2 * H, W + 8], BF16, tag="rhs4p")
            rhs4n = k4p.tile([2 * H, W + 8], BF16, tag="rhs4n")
            nc.scalar.activation(out=k4lhs[0:H, :], in_=a_ps, func=Cp)
            nc.sync.dma_start(
                out=k4lhs[H : 2 * H, :], in_=masks[s, 0:H, 4 : W + 4]
            )
            mab = mk.tile([H, W], BF16, tag="mab")
            nc.sync.dma_start(out=mab, in_=masks[s, 0:H, 4 : W + 4])
            nc.sync.dma_start(out=rhs4p[0:H, :], in_=masks[s, H : 2 * H, :])
            nc.scalar.activation(out=rhs4p[H : 2 * H, 4 : W + 4], in_=b_ps, func=Cp)
            nc.sync.dma_start(out=rhs4n[0:H, :], in_=masks[s, 2 * H : 3 * H, :])
            nc.scalar.activation(out=rhs4n[H : 2 * H, 4 : W + 4], in_=c_ps, func=Cp)
            for r4 in (rhs4p, rhs4n):
                nc.vector.tensor_copy(
                    out=r4[H : 2 * H, 0:4], in_=r4[H : 2 * H, W : W + 4]
                )
                nc.vector.tensor_copy(
                    out=r4[H : 2 * H, W + 4 : W + 8], in_=r4[H : 2 * H, 4:8]
                )

            # ---- Gram matmuls ----
            for pair, (ext, r4) in enumerate(((vext, rhs4p), (wext, rhs4n))):
                num_ps = gram.tile([128, NW], F32, tag="num")
                den_ps = gram.tile([128, NW], F32, tag="den")
                for j1 in range(JB):
                    mb = slice(j1 * 128, (j1 + 1) * 128)
                    wn = slice(j1 * 128, j1 * 128 + NW)
                    for kc in range(NB):
                        nc.tensor.matmul(
                            num_ps, ubuf[:, kc, mb], ext[:, kc, wn],
                            start=(j1 == 0 and kc == 0), stop=False,
                        )
                    nc.tensor.matmul(
                        num_ps, k4lhs[:, mb], r4[:, wn],
                        start=False, stop=(j1 == JB - 1),
                    )
                    nc.tensor.matmul(
                        den_ps, mab[:, mb], r4[0:H, wn],
                        start=(j1 == 0), stop=(j1 == JB - 1),
                    )
                psb = outsb.tile([128, 2, NW], F32, tag="psb")
                nc.scalar.activation(out=psb[:, 0, :], in_=num_ps, func=Cp)
                nc.vector.tensor_copy(out=psb[:, 1, :], in_=den_ps)
                nc.sync.dma_start(out=raw[s, pair], in_=psb)
    if for_hw:
        _split_waits_pass(nc)
    return nc


def _host_prep(a, p, n, ma, mp, mn):
    bf = ml_dtypes.bfloat16
    A = np.ascontiguousarray(a.reshape(B, R, W)).astype(bf)
    P = np.ascontiguousarray(p.reshape(B, R, W)).astype(bf)
    N = np.ascontiguousarray(n.reshape(B, R, W)).astype(bf)
    M0 = np.concatenate(
        [ma.reshape(B, H, W), mp.reshape(B, H, W), mn.reshape(B, H, W)], axis=1
    ).astype(bf)
    M = np.concatenate([M0[:, :, W - 4 :], M0, M0[:, :, :4]], axis=2)
    # replicas: [b, p, t, w] = mask_t[p % 32, w]; mp/mn rows pre-scaled by -2
    Mr = np.stack(
        [
            np.tile(ma.reshape(B, H, W), (1, 4, 1)),
            np.tile(mp.reshape(B, H, W).astype(np.float32) * -2.0, (1, 4, 1)),
            np.tile(mn.reshape(B, H, W).astype(np.float32) * -2.0, (1, 4, 1)),
        ],
        axis=2,
    ).astype(bf)
    ind = np.zeros((128, H), dtype=bf)
    ind[np.arange(128), np.arange(128) % H] = 1
    in_maps = []
    for c in range(NCORES):
        sl = slice(c * S, (c + 1) * S)
        in_maps.append(
            {
                "x_a": A[sl],
                "x_p": P[sl],
                "x_n": N[sl],
                "masks": M[sl],
                "masks_rep": Mr[sl],
                "ind": ind,
            }
        )
    return in_maps


def _host_finish(raw_all):
    # raw_all: [B, 2, 128, 2, NW] float32
    raw64 = raw_all.astype(np.float64)
    nums = raw64[:, :, :, 0]  # [B, 2, 128, NW]
    dens = raw64[:, :, :, 1]
    idx = np.arange(128)
    dists = []
    for off in range(-SHIFT, SHIFT + 1):
        cols = idx + 4 - off
        num = nums[:, :, idx, cols].sum(axis=-1)   # [B, 2]
        cnt = dens[:, :, idx, cols].sum(axis=-1)
        dists.append(num / (C * cnt + 0.001))
    d = np.min(np.stack(dists, axis=0), axis=0)    # [B, 2]
    loss = np.maximum(d[:, 0] - d[:, 1] + MARGIN, 0.0)
    return np.array(loss.mean(), dtype=np.float32)


def kernel(a, p, n, ma, mp, mn):
    global _nc_cache
    from concourse import bass_utils

    if _nc_cache is None:
        _nc_cache = build_nc()
    nc = _nc_cache
    in_maps = _host_prep(a, p, n, ma, mp, mn)
    res = bass_utils.run_bass_kernel_spmd(nc, in_maps, core_ids=list(range(NCORES)))
    raw_all = np.concatenate([res.results[i]["raw"] for i in range(NCORES)], axis=0)
    return _host_finish(raw_all)


# revision 36
# speedup vs baseline: 1.6305x; 1.0610x over previous
"""Bass/Trainium2 kernel for ExtendedTripletLoss (data-parallel over batch).

Math: for each pair (f1,m1),(f2,m2) and shift off in [-4,4]:
  num(off) = sum mask*(f1-f2r)^2 = t1 + t2 - 2*t3
    t1 = corr(A, m2)(off),   A  = sum_c (m1*f1)^2        [32,512]
    t2 = corr(m1, B2)(off),  B2 = sum_c (m2*f2)^2        [32,512]
    t3 = corr(U, V)(off),    U = m1*f1, V = m2*f2        [512,512]
  den(off) = C * corr(m1, m2)(off) + 1e-3
All correlations at 9 lags are computed on TensorE as Gram-block matmuls:
contraction over rows (c,h), w blocked 4x128; rhs uses a +-4 padded copy so
each block's 136-wide window holds all 9 shifted columns. All 4 w-blocks and
all terms accumulate into ONE PSUM tile [128,136]; lag sums are the 9
diagonals col = i + 4 - off, extracted on the host from the DMA'd blocks.
"""

import os
import sys
from contextlib import ExitStack

import numpy as np

for _p in ("/opt/trn_rl_repo", "/root/.axon_site/_ro/trn_rl_repo"):
    if os.path.isdir(_p) and _p not in sys.path:
        sys.path.insert(0, _p)
        break

import ml_dtypes

import concourse.bass as bass
import concourse.mybir as mybir
import concourse.tile as tile
# This environment's walrus_driver allows only ONE sync-wait per instruction,
# while Tile freely aggregates several. Post-pass: move excess waits onto
# freshly inserted same-engine NOPs directly before the instruction.
_MAXW = 1


def _split_waits_pass(nc):
    n = 0
    for fn in nc.m.functions:
        for blk in fn.blocks:
            out = []
            changed = False
            for inst in blk.instructions:
                si = inst.sync_info
                waits = list(si.on_wait) if si is not None else []
                if len(waits) > _MAXW:
                    for i in range(0, len(waits) - _MAXW, _MAXW):
                        nop = mybir.InstNoOp(name=f"{inst.name}-wsplit{i}")
                        nop.engine = inst.engine
                        nop.sync_info = mybir.SyncInfo(
                            on_update=[], on_wait=waits[i : i + _MAXW]
                        )
                        out.append(nop)
                        n += 1
                    si.on_wait = waits[len(waits) - _MAXW :]
                    changed = True
                out.append(inst)
            if changed:
                blk.instructions = out
    return n


BF16 = mybir.dt.bfloat16
F32 = mybir.dt.float32

B, C, H, W = 64, 16, 32, 512
NCORES = 8
S = B // NCORES          # samples per core
R = C * H                # 512 rows in (c,h) contraction dim
NB = R // 128            # 4 partition chunks
JB = W // 128            # 4 w-blocks
NW = 136                 # window width = 128 + 2*4
MARGIN = 0.15
SHIFT = 4

_nc_cache = None


def build_nc(for_hw=True):
    nc = bass.Bass()
    x_a = nc.declare_dram_parameter("x_a", [S, R, W], BF16, isOutput=False)
    x_p = nc.declare_dram_parameter("x_p", [S, R, W], BF16, isOutput=False)
    x_n = nc.declare_dram_parameter("x_n", [S, R, W], BF16, isOutput=False)
    # masks_ext: circularly padded along W: [:, 0:4]=m[:, 508:512],
    # [:, 4:516]=m, [:, 516:520]=m[:, 0:4]; rows = [ma; mp; mn]
    masks = nc.declare_dram_parameter("masks", [S, 3 * H, W + 8], BF16, isOutput=False)
    # mask replicas: [s, p, t, w] = mask_t[p % 32, w], with t=1,2 pre-scaled by -2
    masks_rep = nc.declare_dram_parameter("masks_rep", [S, 128, 3, W], BF16, isOutput=False)
    ind = nc.declare_dram_parameter("ind", [128, H], BF16, isOutput=False)
    # raw[s, i, g, c]: g = (num-ap, num-an, den-ap, den-an)
    raw = nc.declare_dram_parameter("raw", [S, 128, 4, NW], F32, isOutput=True)

    with tile.TileContext(nc) as tc, ExitStack() as ctx:
        const = ctx.enter_context(tc.tile_pool(name="const", bufs=1))
        io = ctx.enter_context(tc.tile_pool(name="io", bufs=3))
        mk = ctx.enter_context(tc.tile_pool(name="mk", bufs=3))
        um = ctx.enter_context(tc.tile_pool(name="um", bufs=2))
        sq = ctx.enter_context(tc.tile_pool(name="sq", bufs=2))
        k4p = ctx.enter_context(tc.tile_pool(name="k4p", bufs=2))
        outsb = ctx.enter_context(tc.tile_pool(name="outsb", bufs=4))
        indps = ctx.enter_context(tc.tile_pool(name="indps", bufs=3, space="PSUM"))
        gram = ctx.enter_context(tc.tile_pool(name="gram", bufs=2, space="PSUM"))

        ind_sb = const.tile([128, H], BF16)
        nc.sync.dma_start(out=ind_sb, in_=ind[:])

        mult = mybir.AluOpType.mult

        for s in range(S):
            # ---- loads ----
            abuf = io.tile([128, NB, W], BF16, tag="abuf")
            pbuf = io.tile([128, NB, W], BF16, tag="pbuf")
            nbuf = io.tile([128, NB, W], BF16, tag="nbuf")
            nc.sync.dma_start(out=abuf, in_=x_a[s].rearrange("(j p) w -> p j w", p=128))
            nc.sync.dma_start(out=pbuf, in_=x_p[s].rearrange("(j p) w -> p j w", p=128))
            nc.sync.dma_start(out=nbuf, in_=x_n[s].rearrange("(j p) w -> p j w", p=128))

            # ---- mask replicas to 128 partitions: one broadcast DMA each ----
            mrep = mk.tile([128, 3, W], BF16, tag="mrep")
            nc.gpsimd.dma_start(out=mrep, in_=masks_rep[s])

            def rep_b(t):
                # [128, NB, W] view of mrep[:, t, :] broadcast over the NB axis
                return mrep[:, t, :].unsqueeze(1).broadcast_to((128, NB, W))

            # ---- masking (DVE): U = ma*a ; vw = [-2*mp*p | -2*mn*n] ----
            ubuf = um.tile([128, NB, W], BF16, tag="ubuf")
            vw = um.tile([128, NB, 2, W + 8], BF16, tag="vw")
            nc.vector.tensor_tensor(out=ubuf, in0=abuf, in1=rep_b(0), op=mult)
            nc.vector.tensor_tensor(
                out=vw[:, :, 0, 4 : W + 4], in0=pbuf, in1=rep_b(1), op=mult
            )
            nc.vector.tensor_tensor(
                out=vw[:, :, 1, 4 : W + 4], in0=nbuf, in1=rep_b(2), op=mult
            )
            # circular wrap columns (both pairs at once)
            nc.vector.tensor_copy(out=vw[:, :, :, 0:4], in_=vw[:, :, :, W : W + 4])
            nc.vector.tensor_copy(out=vw[:, :, :, W + 4 : W + 8], in_=vw[:, :, :, 4:8])

            # ---- squares: u2 on DVE, v2/w2 on ACT ----
            u2 = sq.tile([128, NB, W], BF16, tag="u2")
            v2 = sq.tile([128, NB, W], BF16, tag="v2")
            w2 = sq.tile([128, NB, W], BF16, tag="w2")
            Sq = mybir.ActivationFunctionType.Square
            nc.vector.tensor_tensor(out=u2, in0=ubuf, in1=ubuf, op=mult)
            nc.scalar.activation(out=v2, in_=vw[:, :, 0, 4 : W + 4], func=Sq, scale=0.5)
            nc.scalar.activation(out=w2, in_=vw[:, :, 1, 4 : W + 4], func=Sq, scale=0.5)

            # ---- c-reduction via indicator matmul: A/Bp/Bn [32, 512] ----
            a_ps = indps.tile([H, W], F32, tag="ind3")
            b_ps = indps.tile([H, W], F32, tag="ind3")
            c_ps = indps.tile([H, W], F32, tag="ind3")
            for j in range(NB):
                nc.tensor.matmul(a_ps, ind_sb, u2[:, j, :], start=(j == 0), stop=(j == NB - 1))
            for j in range(NB):
                nc.tensor.matmul(b_ps, ind_sb, v2[:, j, :], start=(j == 0), stop=(j == NB - 1))
            for j in range(NB):
                nc.tensor.matmul(c_ps, ind_sb, w2[:, j, :], start=(j == 0), stop=(j == NB - 1))

            # ---- assemble k4 lhsT [A; ma] and rhs [m2_ext; B2_ext] x pairs ----
            Cp = mybir.ActivationFunctionType.Copy
            k4lhs = k4p.tile([2 * H, W], BF16, tag="k4lhs")
            r44 = k4p.tile([2 * H, 2, W + 8], BF16, tag="r44")
            nc.scalar.activation(out=k4lhs[0:H, :], in_=a_ps, func=Cp)
            nc.sync.dma_start(
                out=k4lhs[H : 2 * H, :], in_=masks[s, 0:H, 4 : W + 4]
            )
            mab = mk.tile([H, W], BF16, tag="mab")
            nc.sync.dma_start(out=mab, in_=masks[s, 0:H, 4 : W + 4])
            nc.sync.dma_start(
                out=r44[0:H, :, :],
                in_=masks[s, H : 3 * H, :].rearrange("(t p) w -> p t w", p=H),
            )
            nc.scalar.activation(out=r44[H : 2 * H, 0, 4 : W + 4], in_=b_ps, func=Cp)
            nc.scalar.activation(out=r44[H : 2 * H, 1, 4 : W + 4], in_=c_ps, func=Cp)
            nc.vector.tensor_copy(
                out=r44[H : 2 * H, :, 0:4], in_=r44[H : 2 * H, :, W : W + 4]
            )
            nc.vector.tensor_copy(
                out=r44[H : 2 * H, :, W + 4 : W + 8], in_=r44[H : 2 * H, :, 4:8]
            )

            # ---- Gram matmuls (both pairs per matmul via 3D rhs) ----
            num_ps = gram.tile([128, 2, NW], F32, tag="num")
            den_ps = gram.tile([128, 2, NW], F32, tag="den")
            for j1 in range(JB):
                mb = slice(j1 * 128, (j1 + 1) * 128)
                wn = slice(j1 * 128, j1 * 128 + NW)
                for kc in range(NB):
                    nc.tensor.matmul(
                        num_ps, ubuf[:, kc, mb], vw[:, kc, :, wn],
                        start=(j1 == 0 and kc == 0), stop=False,
                    )
                nc.tensor.matmul(
                    num_ps, k4lhs[:, mb], r44[:, :, wn],
                    start=False, stop=(j1 == JB - 1),
                )
                nc.tensor.matmul(
                    den_ps, mab[:, mb], r44[0:H, :, wn],
                    start=(j1 == 0), stop=(j1 == JB - 1),
                )
            psb = outsb.tile([128, 4, NW], F32, tag="psb")
            nc.scalar.activation(out=psb[:, 0:2, :], in_=num_ps, func=Cp)
            nc.vector.tensor_copy(out=psb[:, 2:4, :], in_=den_ps)
            nc.sync.dma_start(out=raw[s], in_=psb)
    if for_hw:
        _split_waits_pass(nc)
    return nc


def _host_prep(a, p, n, ma, mp, mn):
    bf = ml_dtypes.bfloat16
    A = np.ascontiguousarray(a.reshape(B, R, W)).astype(bf)
    P = np.ascontiguousarray(p.reshape(B, R, W)).astype(bf)
    N = np.ascontiguousarray(n.reshape(B, R, W)).astype(bf)
    M0 = np.concatenate(
        [ma.reshape(B, H, W), mp.reshape(B, H, W), mn.reshape(B, H, W)], axis=1
    ).astype(bf)
    M = np.concatenate([M0[:, :, W - 4 :], M0, M0[:, :, :4]], axis=2)
    # replicas: [b, p, t, w] = mask_t[p % 32, w]; mp/mn rows pre-scaled by -2
    Mr = np.stack(
        [
            np.tile(ma.reshape(B, H, W), (1, 4, 1)),
            np.tile(mp.reshape(B, H, W).astype(np.float32) * -2.0, (1, 4, 1)),
            np.tile(mn.reshape(B, H, W).astype(np.float32) * -2.0, (1, 4, 1)),
        ],
        axis=2,
    ).astype(bf)
    ind = np.zeros((128, H), dtype=bf)
    ind[np.arange(128), np.arange(128) % H] = 1
    in_maps = []
    for c in range(NCORES):
        sl = slice(c * S, (c + 1) * S)
        in_maps.append(
            {
                "x_a": A[sl],
                "x_p": P[sl],
                "x_n": N[sl],
                "masks": M[sl],
                "masks_rep": Mr[sl],
                "ind": ind,
            }
        )
    return in_maps


def _host_finish(raw_all):
    # raw_all: [B, 128, 4, NW] float32; g = (num-ap, num-an, den-ap, den-an)
    raw64 = raw_all.astype(np.float64)
    nums = raw64[:, :, 0:2].transpose(0, 2, 1, 3)  # [B, 2, 128, NW]
    dens = raw64[:, :, 2:4].transpose(0, 2, 1, 3)
    idx = np.arange(128)
    dists = []
    for off in range(-SHIFT, SHIFT + 1):
        cols = idx + 4 - off
        num = nums[:, :, idx, cols].sum(axis=-1)   # [B, 2]
        cnt = dens[:, :, idx, cols].sum(axis=-1)
        dists.append(num / (C * cnt + 0.001))
    d = np.min(np.stack(dists, axis=0), axis=0)    # [B, 2]
    loss = np.maximum(d[:, 0] - d[:, 1] + MARGIN, 0.0)
    return np.array(loss.mean(), dtype=np.float32)


def kernel(a, p, n, ma, mp, mn):
    global _nc_cache
    from concourse import bass_utils

    if _nc_cache is None:
        _nc_cache = build_nc()
    nc = _nc_cache
    in_maps = _host_prep(a, p, n, ma, mp, mn)
    res = bass_utils.run_bass_kernel_spmd(nc, in_maps, core_ids=list(range(NCORES)))
    raw_all = np.concatenate([res.results[i]["raw"] for i in range(NCORES)], axis=0)
    return _host_finish(raw_all)


# revision 41
# speedup vs baseline: 1.7693x; 1.0852x over previous
"""Bass/Trainium2 kernel for ExtendedTripletLoss (data-parallel over batch).

Math: for each pair (f1,m1),(f2,m2) and shift off in [-4,4]:
  num(off) = sum mask*(f1-f2r)^2 = t1 + t2 - 2*t3
    t1 = corr(A, m2)(off),   A  = sum_c (m1*f1)^2        [32,512]
    t2 = corr(m1, B2)(off),  B2 = sum_c (m2*f2)^2        [32,512]
    t3 = corr(U, V)(off),    U = m1*f1, V = m2*f2        [512,512]
  den(off) = C * corr(m1, m2)(off) + 1e-3
All correlations at 9 lags are computed on TensorE as Gram-block matmuls:
contraction over rows (c,h), w blocked 4x128; rhs uses a +-4 padded copy so
each block's 136-wide window holds all 9 shifted columns. All 4 w-blocks and
all terms accumulate into ONE PSUM tile [128,136]; lag sums are the 9
diagonals col = i + 4 - off, extracted on the host from the DMA'd blocks.
"""

import os
import sys
from contextlib import ExitStack

import numpy as np

for _p in ("/opt/trn_rl_repo", "/root/.axon_site/_ro/trn_rl_repo"):
    if os.path.isdir(_p) and _p not in sys.path:
        sys.path.insert(0, _p)
        break

import ml_dtypes

import concourse.bass as bass
import concourse.mybir as mybir
import concourse.tile as tile
# This environment's walrus_driver allows only ONE sync-wait per instruction,
# while Tile freely aggregates several. Post-pass: move excess waits onto
# freshly inserted same-engine NOPs directly before the instruction.
_MAXW = 1


def _split_waits_pass(nc):
    n = 0
    for fn in nc.m.functions:
        for blk in fn.blocks:
            out = []
            changed = False
            for inst in blk.instructions:
                si = inst.sync_info
                waits = list(si.on_wait) if si is not None else []
                if len(waits) > _MAXW:
                    for i in range(0, len(waits) - _MAXW, _MAXW):
                        nop = mybir.InstNoOp(name=f"{inst.name}-wsplit{i}")
                        nop.engine = inst.engine
                        nop.sync_info = mybir.SyncInfo(
                            on_update=[], on_wait=waits[i : i + _MAXW]
                        )
                        out.append(nop)
                        n += 1
                    si.on_wait = waits[len(waits) - _MAXW :]
                    changed = True
                out.append(inst)
            if changed:
                blk.instructions = out
    return n


BF16 = mybir.dt.bfloat16
F32 = mybir.dt.float32

B, C, H, W = 64, 16, 32, 512
NCORES = 8
S = B // NCORES          # samples per core
R = C * H                # 512 rows in (c,h) contraction dim
NB = R // 128            # 4 partition chunks
JB = W // 128            # 4 w-blocks
NW = 136                 # window width = 128 + 2*4
MARGIN = 0.15
SHIFT = 4

_nc_cache = None


def build_nc(for_hw=True):
    nc = bass.Bass()
    x_a = nc.declare_dram_parameter("x_a", [S, R, W], BF16, isOutput=False)
    x_p = nc.declare_dram_parameter("x_p", [S, R, W], BF16, isOutput=False)
    x_n = nc.declare_dram_parameter("x_n", [S, R, W], BF16, isOutput=False)
    # masks_ext: circularly padded along W: [:, 0:4]=m[:, 508:512],
    # [:, 4:516]=m, [:, 516:520]=m[:, 0:4]; rows = [ma; mp; mn]
    masks = nc.declare_dram_parameter("masks", [S, 3 * H, W + 8], BF16, isOutput=False)
    # mask replicas: [s, p, t, w] = mask_t[p % 32, w], with t=1,2 pre-scaled by -2
    masks_rep = nc.declare_dram_parameter("masks_rep", [S, 128, 3, W], BF16, isOutput=False)
    ind = nc.declare_dram_parameter("ind", [128, H], BF16, isOutput=False)
    # raw[s, i, g, c]: g = (num-ap, num-an, den-ap, den-an)
    raw = nc.declare_dram_parameter("raw", [S, 128, 4, NW], F32, isOutput=True)

    with tile.TileContext(nc) as tc, ExitStack() as ctx:
        const = ctx.enter_context(tc.tile_pool(name="const", bufs=1))
        io = ctx.enter_context(tc.tile_pool(name="io", bufs=3))
        mk = ctx.enter_context(tc.tile_pool(name="mk", bufs=3))
        um = ctx.enter_context(tc.tile_pool(name="um", bufs=3))
        sq = ctx.enter_context(tc.tile_pool(name="sq", bufs=3))
        k4p = ctx.enter_context(tc.tile_pool(name="k4p", bufs=3))
        outsb = ctx.enter_context(tc.tile_pool(name="outsb", bufs=4))
        indps = ctx.enter_context(tc.tile_pool(name="indps", bufs=3, space="PSUM"))
        gram = ctx.enter_context(tc.tile_pool(name="gram", bufs=2, space="PSUM"))

        ind_sb = const.tile([128, H], BF16)
        nc.sync.dma_start(out=ind_sb, in_=ind[:])

        mult = mybir.AluOpType.mult

        for s in range(S):
            # ---- loads ----
            abuf = io.tile([128, NB, W], BF16, tag="abuf")
            pn = io.tile([128, NB, 2, W], BF16, tag="pn")
            nc.sync.dma_start(out=abuf, in_=x_a[s].rearrange("(j p) w -> p j w", p=128))
            nc.sync.dma_start(
                out=pn[:, :, 0, :], in_=x_p[s].rearrange("(j p) w -> p j w", p=128)
            )
            nc.sync.dma_start(
                out=pn[:, :, 1, :], in_=x_n[s].rearrange("(j p) w -> p j w", p=128)
            )

            # ---- mask replicas to 128 partitions: one broadcast DMA each ----
            mrep = mk.tile([128, 3, W], BF16, tag="mrep")
            nc.gpsimd.dma_start(out=mrep, in_=masks_rep[s])

            def rep_b(t):
                # [128, NB, W] view of mrep[:, t, :] broadcast over the NB axis
                return mrep[:, t, :].unsqueeze(1).broadcast_to((128, NB, W))

            # ---- masking (DVE): U = ma*a ; vw = [-2*mp*p | -2*mn*n] ----
            ubuf = um.tile([128, NB, W], BF16, tag="ubuf")
            vw = um.tile([128, NB, 2, W + 8], BF16, tag="vw")
            nc.vector.tensor_tensor(out=ubuf, in0=abuf, in1=rep_b(0), op=mult)
            nc.vector.tensor_tensor(
                out=vw[:, :, :, 4 : W + 4],
                in0=pn,
                in1=mrep[:, 1:3, :].unsqueeze(1).broadcast_to((128, NB, 2, W)),
                op=mult,
            )
            # circular wrap columns (both pairs at once)
            nc.vector.tensor_copy(out=vw[:, :, :, 0:4], in_=vw[:, :, :, W : W + 4])
            nc.vector.tensor_copy(out=vw[:, :, :, W + 4 : W + 8], in_=vw[:, :, :, 4:8])

            # ---- squares: u2 on DVE, v2/w2 on ACT (one op) ----
            u2 = sq.tile([128, NB, W], BF16, tag="u2")
            vw2 = sq.tile([128, NB, 2, W], BF16, tag="vw2")
            Sq = mybir.ActivationFunctionType.Square
            nc.vector.tensor_tensor(out=u2, in0=ubuf, in1=ubuf, op=mult)
            nc.scalar.activation(out=vw2, in_=vw[:, :, :, 4 : W + 4], func=Sq, scale=0.5)

            # ---- c-reduction via indicator matmul: A/Bp/Bn [32, 512] ----
            a_ps = indps.tile([H, W], F32, tag="ind3")
            b_ps = indps.tile([H, W], F32, tag="ind3")
            c_ps = indps.tile([H, W], F32, tag="ind3")
            for j in range(NB):
                nc.tensor.matmul(a_ps, ind_sb, u2[:, j, :], start=(j == 0), stop=(j == NB - 1))
            for j in range(NB):
                nc.tensor.matmul(b_ps, ind_sb, vw2[:, j, 0, :], start=(j == 0), stop=(j == NB - 1))
            for j in range(NB):
                nc.tensor.matmul(c_ps, ind_sb, vw2[:, j, 1, :], start=(j == 0), stop=(j == NB - 1))

            # ---- assemble k4 lhsT [A; ma] and rhs [m2_ext; B2_ext] x pairs ----
            Cp = mybir.ActivationFunctionType.Copy
            k4lhs = k4p.tile([2 * H, W], BF16, tag="k4lhs")
            r44 = k4p.tile([2 * H, 2, W + 8], BF16, tag="r44")
            nc.scalar.activation(out=k4lhs[0:H, :], in_=a_ps, func=Cp)
            nc.gpsimd.dma_start(
                out=k4lhs[H : 2 * H, :], in_=masks[s, 0:H, 4 : W + 4]
            )
            mab = mk.tile([H, W], BF16, tag="mab")
            nc.gpsimd.dma_start(out=mab, in_=masks[s, 0:H, 4 : W + 4])
            nc.gpsimd.dma_start(
                out=r44[0:H, :, :],
                in_=masks[s, H : 3 * H, :].rearrange("(t p) w -> p t w", p=H),
            )
            nc.scalar.activation(out=r44[H : 2 * H, 0, 4 : W + 4], in_=b_ps, func=Cp)
            nc.scalar.activation(out=r44[H : 2 * H, 1, 4 : W + 4], in_=c_ps, func=Cp)
            nc.vector.tensor_copy(
                out=r44[H : 2 * H, :, 0:4], in_=r44[H : 2 * H, :, W : W + 4]
            )
            nc.vector.tensor_copy(
                out=r44[H : 2 * H, :, W + 4 : W + 8], in_=r44[H : 2 * H, :, 4:8]
            )

            # ---- Gram matmuls (both pairs per matmul via 3D rhs) ----
            num_ps = gram.tile([128, 2, NW], F32, tag="num")
            den_ps = gram.tile([128, 2, NW], F32, tag="den")
            for j1 in range(JB):
                mb = slice(j1 * 128, (j1 + 1) * 128)
                wn = slice(j1 * 128, j1 * 128 + NW)
                for kc in range(NB):
                    nc.tensor.matmul(
                        num_ps, ubuf[:, kc, mb], vw[:, kc, :, wn],
                        start=(j1 == 0 and kc == 0), stop=False,
                    )
                nc.tensor.matmul(
                    num_ps, k4lhs[:, mb], r44[:, :, wn],
                    start=False, stop=(j1 == JB - 1),
                )
                nc.tensor.matmul(
                    den_ps, mab[:, mb], r44[0:H, :, wn],
                    start=(j1 == 0), stop=(j1 == JB - 1),
                )
            psb = outsb.tile([128, 4, NW], F32, tag="psb")
            nc.scalar.activation(out=psb[:, 0:2, :], in_=num_ps, func=Cp)
            nc.vector.tensor_copy(out=psb[:, 2:4, :], in_=den_ps)
            nc.sync.dma_start(out=raw[s], in_=psb)
    if for_hw:
        _split_waits_pass(nc)
    return nc


def _host_prep(a, p, n, ma, mp, mn):
    bf = ml_dtypes.bfloat16
    A = np.ascontiguousarray(a.reshape(B, R, W)).astype(bf)
    P = np.ascontiguousarray(p.reshape(B, R, W)).astype(bf)
    N = np.ascontiguousarray(n.reshape(B, R, W)).astype(bf)
    M0 = np.concatenate(
        [ma.reshape(B, H, W), mp.reshape(B, H, W), mn.reshape(B, H, W)], axis=1
    ).astype(bf)
    M = np.concatenate([M0[:, :, W - 4 :], M0, M0[:, :, :4]], axis=2)
    # replicas: [b, p, t, w] = mask_t[p % 32, w]; mp/mn rows pre-scaled by -2
    Mr = np.stack(
        [
            np.tile(ma.reshape(B, H, W), (1, 4, 1)),
            np.tile(mp.reshape(B, H, W).astype(np.float32) * -2.0, (1, 4, 1)),
            np.tile(mn.reshape(B, H, W).astype(np.float32) * -2.0, (1, 4, 1)),
        ],
        axis=2,
    ).astype(bf)
    ind = np.zeros((128, H), dtype=bf)
    ind[np.arange(128), np.arange(128) % H] = 1
    in_maps = []
    for c in range(NCORES):
        sl = slice(c * S, (c + 1) * S)
        in_maps.append(
            {
                "x_a": A[sl],
                "x_p": P[sl],
                "x_n": N[sl],
                "masks": M[sl],
                "masks_rep": Mr[sl],
                "ind": ind,
            }
        )
    return in_maps


def _host_finish(raw_all):
    # raw_all: [B, 128, 4, NW] float32; g = (num-ap, num-an, den-ap, den-an)
    raw64 = raw_all.astype(np.float64)
    nums = raw64[:, :, 0:2].transpose(0, 2, 1, 3)  # [B, 2, 128, NW]
    dens = raw64[:, :, 2:4].transpose(0, 2, 1, 3)
    idx = np.arange(128)
    dists = []
    for off in range(-SHIFT, SHIFT + 1):
        cols = idx + 4 - off
        num = nums[:, :, idx, cols].sum(axis=-1)   # [B, 2]
        cnt = dens[:, :, idx, cols].sum(axis=-1)
        dists.append(num / (C * cnt + 0.001))
    d = np.min(np.stack(dists, axis=0), axis=0)    # [B, 2]
    loss = np.maximum(d[:, 0] - d[:, 1] + MARGIN, 0.0)
    return np.array(loss.mean(), dtype=np.float32)


def kernel(a, p, n, ma, mp, mn):
    global _nc_cache
    from concourse import bass_utils

    if _nc_cache is None:
        _nc_cache = build_nc()
    nc = _nc_cache
    in_maps = _host_prep(a, p, n, ma, mp, mn)
    res = bass_utils.run_bass_kernel_spmd(nc, in_maps, core_ids=list(range(NCORES)))
    raw_all = np.concatenate([res.results[i]["raw"] for i in range(NCORES)], axis=0)
    return _host_finish(raw_all)


# revision 50
# speedup vs baseline: 1.7864x; 1.0097x over previous
"""Bass/Trainium2 kernel for ExtendedTripletLoss (data-parallel over batch).

Math: for each pair (f1,m1),(f2,m2) and shift off in [-4,4]:
  num(off) = sum mask*(f1-f2r)^2 = t1 + t2 - 2*t3
    t1 = corr(A, m2)(off),   A  = sum_c (m1*f1)^2        [32,512]
    t2 = corr(m1, B2)(off),  B2 = sum_c (m2*f2)^2        [32,512]
    t3 = corr(U, V)(off),    U = m1*f1, V = m2*f2        [512,512]
  den(off) = C * corr(m1, m2)(off) + 1e-3
All correlations at 9 lags are computed on TensorE as Gram-block matmuls:
contraction over rows (c,h), w blocked 4x128; rhs uses a +-4 padded copy so
each block's 136-wide window holds all 9 shifted columns. All 4 w-blocks and
all terms accumulate into ONE PSUM tile [128,136]; lag sums are the 9
diagonals col = i + 4 - off, extracted on the host from the DMA'd blocks.
"""

import os
import sys
from contextlib import ExitStack

import numpy as np

for _p in ("/opt/trn_rl_repo", "/root/.axon_site/_ro/trn_rl_repo"):
    if os.path.isdir(_p) and _p not in sys.path:
        sys.path.insert(0, _p)
        break

import ml_dtypes

import concourse.bass as bass
import concourse.mybir as mybir
import concourse.tile as tile
# This environment's walrus_driver allows only ONE sync-wait per instruction,
# while Tile freely aggregates several. Post-pass: move excess waits onto
# freshly inserted same-engine NOPs directly before the instruction.
_MAXW = 1


def _split_waits_pass(nc):
    n = 0
    for fn in nc.m.functions:
        for blk in fn.blocks:
            out = []
            changed = False
            for inst in blk.instructions:
                si = inst.sync_info
                waits = list(si.on_wait) if si is not None else []
                if len(waits) > _MAXW:
                    for i in range(0, len(waits) - _MAXW, _MAXW):
                        nop = mybir.InstNoOp(name=f"{inst.name}-wsplit{i}")
                        nop.engine = inst.engine
                        nop.sync_info = mybir.SyncInfo(
                            on_update=[], on_wait=waits[i : i + _MAXW]
                        )
                        out.append(nop)
                        n += 1
                    si.on_wait = waits[len(waits) - _MAXW :]
                    changed = True
                out.append(inst)
            if changed:
                blk.instructions = out
    return n


# concourse pins --enable-ldw-opt=false; enabling lets walrus elide/overlap
# redundant weight loads, which are ~30% of this kernel's PE time.
def _patch_ldw_opt():
    from concourse import bass_utils as _bu

    if getattr(_bu, "_ldw_opt_patched", False):
        return
    _orig = _bu.run_command

    def _run_command_ldwopt(cmd, *a, **kw):
        if isinstance(cmd, list):
            cmd = [
                "--enable-ldw-opt=true" if c == "--enable-ldw-opt=false" else c
                for c in cmd
            ]
        return _orig(cmd, *a, **kw)

    _bu.run_command = _run_command_ldwopt
    _bu._ldw_opt_patched = True


if os.environ.get("BASS_LDW_OPT", "0") == "1":
    _patch_ldw_opt()

BF16 = mybir.dt.bfloat16
F32 = mybir.dt.float32

B, C, H, W = 64, 16, 32, 512
NCORES = 8
S = B // NCORES          # samples per core
R = C * H                # 512 rows in (c,h) contraction dim
NB = R // 128            # 4 partition chunks
JB = W // 128            # 4 w-blocks
NW = 136                 # window width = 128 + 2*4
MARGIN = 0.15
SHIFT = 4

_nc_cache = None


def build_nc(for_hw=True):
    nc = bass.Bass()
    x_a = nc.declare_dram_parameter("x_a", [S, R, W], BF16, isOutput=False)
    x_p = nc.declare_dram_parameter("x_p", [S, R, W], BF16, isOutput=False)
    x_n = nc.declare_dram_parameter("x_n", [S, R, W], BF16, isOutput=False)
    # masks_ext: circularly padded along W: [:, 0:4]=m[:, 508:512],
    # [:, 4:516]=m, [:, 516:520]=m[:, 0:4]; rows = [ma; mp; mn]
    masks = nc.declare_dram_parameter("masks", [S, 3 * H, W + 8], BF16, isOutput=False)
    # mask replicas: [s, p, t, w] = mask_t[p % 32, w], with t=1,2 pre-scaled by -2
    masks_rep = nc.declare_dram_parameter("masks_rep", [S, 128, 3, W], BF16, isOutput=False)
    ind = nc.declare_dram_parameter("ind", [128, H], BF16, isOutput=False)
    # raw[s, i, g, c]: g = (num-ap, num-an); den is host-computed from masks
    raw = nc.declare_dram_parameter("raw", [S, 128, 2, NW], F32, isOutput=True)

    with tile.TileContext(nc) as tc, ExitStack() as ctx:
        const = ctx.enter_context(tc.tile_pool(name="const", bufs=1))
        io = ctx.enter_context(tc.tile_pool(name="io", bufs=3))
        mk = ctx.enter_context(tc.tile_pool(name="mk", bufs=3))
        um = ctx.enter_context(tc.tile_pool(name="um", bufs=3))
        sq = ctx.enter_context(tc.tile_pool(name="sq", bufs=3))
        k4p = ctx.enter_context(tc.tile_pool(name="k4p", bufs=3))
        outsb = ctx.enter_context(tc.tile_pool(name="outsb", bufs=4))
        indps = ctx.enter_context(tc.tile_pool(name="indps", bufs=3, space="PSUM"))
        gram = ctx.enter_context(tc.tile_pool(name="gram", bufs=2, space="PSUM"))

        ind_sb = const.tile([128, H], BF16)
        nc.sync.dma_start(out=ind_sb, in_=ind[:])

        # PE prewarm: ~4us of junk matmuls so the HAM un-throttles during
        # the pipeline-fill phase instead of during the first real samples.
        warm_ps = ctx.enter_context(
            tc.tile_pool(name="warm", bufs=1, space="PSUM")
        ).tile([H, H], F32)
        for _ in range(60):
            nc.tensor.matmul(warm_ps, ind_sb, ind_sb[:, 0:H], start=True, stop=True)

        mult = mybir.AluOpType.mult

        for s in range(S):
            # ---- loads ----
            abuf = io.tile([128, NB, W], BF16, tag="abuf")
            pn = io.tile([128, NB, 2, W], BF16, tag="pn")
            nc.sync.dma_start(out=abuf, in_=x_a[s].rearrange("(j p) w -> p j w", p=128))
            nc.sync.dma_start(
                out=pn[:, :, 0, :], in_=x_p[s].rearrange("(j p) w -> p j w", p=128)
            )
            nc.sync.dma_start(
                out=pn[:, :, 1, :], in_=x_n[s].rearrange("(j p) w -> p j w", p=128)
            )

            # ---- mask replicas to 128 partitions: one broadcast DMA each ----
            mrep = mk.tile([128, 3, W], BF16, tag="mrep")
            nc.gpsimd.dma_start(out=mrep, in_=masks_rep[s])

            def rep_b(t):
                # [128, NB, W] view of mrep[:, t, :] broadcast over the NB axis
                return mrep[:, t, :].unsqueeze(1).broadcast_to((128, NB, W))

            # ---- masking (DVE): U = ma*a ; vw = [-2*mp*p | -2*mn*n] ----
            ubuf = um.tile([128, NB, W], BF16, tag="ubuf")
            vw = um.tile([128, NB, 2, W + 8], BF16, tag="vw")
            nc.vector.tensor_tensor(out=ubuf, in0=abuf, in1=rep_b(0), op=mult)
            nc.vector.tensor_tensor(
                out=vw[:, :, :, 4 : W + 4],
                in0=pn,
                in1=mrep[:, 1:3, :].unsqueeze(1).broadcast_to((128, NB, 2, W)),
                op=mult,
            )
            # circular wrap columns (both pairs at once)
            nc.vector.tensor_copy(out=vw[:, :, :, 0:4], in_=vw[:, :, :, W : W + 4])
            nc.vector.tensor_copy(out=vw[:, :, :, W + 4 : W + 8], in_=vw[:, :, :, 4:8])

            # ---- squares: u2 on DVE, v2/w2 on ACT (one op) ----
            u2 = sq.tile([128, NB, W], BF16, tag="u2")
            vw2 = sq.tile([128, NB, 2, W], BF16, tag="vw2")
            Sq = mybir.ActivationFunctionType.Square
            nc.vector.tensor_tensor(out=u2, in0=ubuf, in1=ubuf, op=mult)
            nc.scalar.activation(out=vw2, in_=vw[:, :, :, 4 : W + 4], func=Sq, scale=0.5)

            # ---- c-reduction via indicator matmul: A/Bp/Bn [32, 512] ----
            a_ps = indps.tile([H, W], F32, tag="ind3")
            b_ps = indps.tile([H, W], F32, tag="ind3")
            c_ps = indps.tile([H, W], F32, tag="ind3")
            for j in range(NB):
                nc.tensor.matmul(a_ps, ind_sb, u2[:, j, :], start=(j == 0), stop=(j == NB - 1))
            for j in range(NB):
                nc.tensor.matmul(b_ps, ind_sb, vw2[:, j, 0, :], start=(j == 0), stop=(j == NB - 1))
            for j in range(NB):
                nc.tensor.matmul(c_ps, ind_sb, vw2[:, j, 1, :], start=(j == 0), stop=(j == NB - 1))

            # ---- assemble k4 lhsT [A; ma] and rhs [m2_ext; B2_ext] x pairs ----
            Cp = mybir.ActivationFunctionType.Copy
            k4lhs = k4p.tile([2 * H, W], BF16, tag="k4lhs")
            r44 = k4p.tile([2 * H, 2, W + 8], BF16, tag="r44")
            nc.scalar.activation(out=k4lhs[0:H, :], in_=a_ps, func=Cp)
            nc.gpsimd.dma_start(
                out=k4lhs[H : 2 * H, :], in_=masks[s, 0:H, 4 : W + 4]
            )
            nc.gpsimd.dma_start(
                out=r44[0:H, :, :],
                in_=masks[s, H : 3 * H, :].rearrange("(t p) w -> p t w", p=H),
            )
            nc.scalar.activation(out=r44[H : 2 * H, 0, 4 : W + 4], in_=b_ps, func=Cp)
            nc.scalar.activation(out=r44[H : 2 * H, 1, 4 : W + 4], in_=c_ps, func=Cp)
            nc.vector.tensor_copy(
                out=r44[H : 2 * H, :, 0:4], in_=r44[H : 2 * H, :, W : W + 4]
            )
            nc.vector.tensor_copy(
                out=r44[H : 2 * H, :, W + 4 : W + 8], in_=r44[H : 2 * H, :, 4:8]
            )

            # ---- Gram matmuls (both pairs per matmul via 3D rhs) ----
            num_ps = gram.tile([128, 2, NW], F32, tag="num")
            for j1 in range(JB):
                mb = slice(j1 * 128, (j1 + 1) * 128)
                wn = slice(j1 * 128, j1 * 128 + NW)
                for kc in range(NB):
                    nc.tensor.matmul(
                        num_ps, ubuf[:, kc, mb], vw[:, kc, :, wn],
                        start=(j1 == 0 and kc == 0), stop=False,
                    )
                nc.tensor.matmul(
                    num_ps, k4lhs[:, mb], r44[:, :, wn],
                    start=False, stop=(j1 == JB - 1),
                )
            psb = outsb.tile([128, 2, NW], F32, tag="psb")
            nc.scalar.activation(out=psb, in_=num_ps, func=Cp)
            nc.sync.dma_start(out=raw[s], in_=psb)
    if for_hw:
        _split_waits_pass(nc)
    return nc


def _host_prep(a, p, n, ma, mp, mn):
    bf = ml_dtypes.bfloat16
    A = np.ascontiguousarray(a.reshape(B, R, W)).astype(bf)
    P = np.ascontiguousarray(p.reshape(B, R, W)).astype(bf)
    N = np.ascontiguousarray(n.reshape(B, R, W)).astype(bf)
    M0 = np.concatenate(
        [ma.reshape(B, H, W), mp.reshape(B, H, W), mn.reshape(B, H, W)], axis=1
    ).astype(bf)
    M = np.concatenate([M0[:, :, W - 4 :], M0, M0[:, :, :4]], axis=2)
    # replicas: [b, p, t, w] = mask_t[p % 32, w]; mp/mn rows pre-scaled by -2
    Mr = np.stack(
        [
            np.tile(ma.reshape(B, H, W), (1, 4, 1)),
            np.tile(mp.reshape(B, H, W).astype(np.float32) * -2.0, (1, 4, 1)),
            np.tile(mn.reshape(B, H, W).astype(np.float32) * -2.0, (1, 4, 1)),
        ],
        axis=2,
    ).astype(bf)
    ind = np.zeros((128, H), dtype=bf)
    ind[np.arange(128), np.arange(128) % H] = 1
    in_maps = []
    for c in range(NCORES):
        sl = slice(c * S, (c + 1) * S)
        in_maps.append(
            {
                "x_a": A[sl],
                "x_p": P[sl],
                "x_n": N[sl],
                "masks": M[sl],
                "masks_rep": Mr[sl],
                "ind": ind,
            }
        )
    return in_maps


def _host_den(ma, mp, mn):
    # den counts[b, pair, off] = sum(m1 & roll(m2, off, -1)) over (1,2,3)
    nb = ma.shape[0]
    m1 = ma.reshape(nb, H, W).astype(bool)
    cnts = np.empty((nb, 2, 2 * SHIFT + 1), np.float64)
    for pair, m2 in enumerate((mp, mn)):
        m2 = m2.reshape(nb, H, W).astype(bool)
        for i, off in enumerate(range(-SHIFT, SHIFT + 1)):
            cnts[:, pair, i] = (m1 & np.roll(m2, off, axis=-1)).sum(axis=(1, 2))
    return cnts


def _host_finish(raw_all, cnts):
    # raw_all: [B, 128, 2, NW] float32; g = (num-ap, num-an)
    raw64 = raw_all.astype(np.float64)
    nums = raw64.transpose(0, 2, 1, 3)             # [B, 2, 128, NW]
    idx = np.arange(128)
    dists = []
    for i, off in enumerate(range(-SHIFT, SHIFT + 1)):
        cols = idx + 4 - off
        num = nums[:, :, idx, cols].sum(axis=-1)   # [B, 2]
        dists.append(num / (C * cnts[:, :, i] + 0.001))
    d = np.min(np.stack(dists, axis=0), axis=0)    # [B, 2]
    loss = np.maximum(d[:, 0] - d[:, 1] + MARGIN, 0.0)
    return np.array(loss.mean(), dtype=np.float32)


def kernel(a, p, n, ma, mp, mn):
    global _nc_cache
    from concourse import bass_utils

    if _nc_cache is None:
        _nc_cache = build_nc()
    nc = _nc_cache
    in_maps = _host_prep(a, p, n, ma, mp, mn)
    res = bass_utils.run_bass_kernel_spmd(nc, in_maps, core_ids=list(range(NCORES)))
    raw_all = np.concatenate([res.results[i]["raw"] for i in range(NCORES)], axis=0)
    return _host_finish(raw_all, _host_den(ma, mp, mn))


# revision 57
# speedup vs baseline: 1.8602x; 1.0413x over previous
"""Bass/Trainium2 kernel for ExtendedTripletLoss (data-parallel over batch).

Math: for each pair (f1,m1),(f2,m2) and shift off in [-4,4]:
  num(off) = sum mask*(f1-f2r)^2 = t1 + t2 - 2*t3
    t1 = corr(A, m2)(off),   A  = sum_c (m1*f1)^2        [32,512]
    t2 = corr(m1, B2)(off),  B2 = sum_c (m2*f2)^2        [32,512]
    t3 = corr(U, V)(off),    U = m1*f1, V = m2*f2        [512,512]
  den(off) = C * corr(m1, m2)(off) + 1e-3
All correlations at 9 lags are computed on TensorE as Gram-block matmuls:
contraction over rows (c,h), w blocked 4x128; rhs uses a +-4 padded copy so
each block's 136-wide window holds all 9 shifted columns. All 4 w-blocks and
all terms accumulate into ONE PSUM tile [128,136]; lag sums are the 9
diagonals col = i + 4 - off, extracted on the host from the DMA'd blocks.
"""

import os
import sys
from contextlib import ExitStack

import numpy as np

for _p in ("/opt/trn_rl_repo", "/root/.axon_site/_ro/trn_rl_repo"):
    if os.path.isdir(_p) and _p not in sys.path:
        sys.path.insert(0, _p)
        break

import ml_dtypes

import concourse.bass as bass
import concourse.mybir as mybir
import concourse.tile as tile
# This environment's walrus_driver allows only ONE sync-wait per instruction,
# while Tile freely aggregates several. Post-pass: move excess waits onto
# freshly inserted same-engine NOPs directly before the instruction.
_MAXW = 1


def _split_waits_pass(nc):
    n = 0
    for fn in nc.m.functions:
        for blk in fn.blocks:
            out = []
            changed = False
            for inst in blk.instructions:
                si = inst.sync_info
                waits = list(si.on_wait) if si is not None else []
                if len(waits) > _MAXW:
                    for i in range(0, len(waits) - _MAXW, _MAXW):
                        nop = mybir.InstNoOp(name=f"{inst.name}-wsplit{i}")
                        nop.engine = inst.engine
                        nop.sync_info = mybir.SyncInfo(
                            on_update=[], on_wait=waits[i : i + _MAXW]
                        )
                        out.append(nop)
                        n += 1
                    si.on_wait = waits[len(waits) - _MAXW :]
                    changed = True
                out.append(inst)
            if changed:
                blk.instructions = out
    return n


# concourse pins --enable-ldw-opt=false; enabling lets walrus elide/overlap
# redundant weight loads, which are ~30% of this kernel's PE time.
def _patch_ldw_opt():
    from concourse import bass_utils as _bu

    if getattr(_bu, "_ldw_opt_patched", False):
        return
    _orig = _bu.run_command

    def _run_command_ldwopt(cmd, *a, **kw):
        if isinstance(cmd, list):
            cmd = [
                "--enable-ldw-opt=true" if c == "--enable-ldw-opt=false" else c
                for c in cmd
            ]
        return _orig(cmd, *a, **kw)

    _bu.run_command = _run_command_ldwopt
    _bu._ldw_opt_patched = True


if os.environ.get("BASS_LDW_OPT", "0") == "1":
    _patch_ldw_opt()

BF16 = mybir.dt.bfloat16
F32 = mybir.dt.float32

B, C, H, W = 64, 16, 32, 512
NCORES = 8
S = B // NCORES          # samples per core
R = C * H                # 512 rows in (c,h) contraction dim
NB = R // 128            # 4 partition chunks
JB = W // 128            # 4 w-blocks
NW = 136                 # window width = 128 + 2*4
MARGIN = 0.15
SHIFT = 4

_nc_cache = None


def build_nc(for_hw=True):
    nc = bass.Bass()
    x_a = nc.declare_dram_parameter("x_a", [S, R, W], BF16, isOutput=False)
    x_p = nc.declare_dram_parameter("x_p", [S, R, W], BF16, isOutput=False)
    x_n = nc.declare_dram_parameter("x_n", [S, R, W], BF16, isOutput=False)
    # masks_ext: circularly padded along W: [:, 0:4]=m[:, 508:512],
    # [:, 4:516]=m, [:, 516:520]=m[:, 0:4]; rows = [ma; mp; mn]
    masks = nc.declare_dram_parameter("masks", [S, 3 * H, W + 8], BF16, isOutput=False)
    # mask replicas: [s, p, t, w] = mask_t[p % 32, w], with t=1,2 pre-scaled by -2
    masks_rep = nc.declare_dram_parameter("masks_rep", [S, 128, 3, W], BF16, isOutput=False)
    ind = nc.declare_dram_parameter("ind", [128, H], BF16, isOutput=False)
    # 0.25-scaled indicator: folds the (-2)^2 of the pre-scaled masks out of
    # the Bp/Bn channel-reductions (exact: power of two)
    ind4 = nc.declare_dram_parameter("ind4", [128, H], BF16, isOutput=False)
    # raw[s, i, g, c]: g = (num-ap, num-an); den is host-computed from masks
    raw = nc.declare_dram_parameter("raw", [S, 128, 2, NW], F32, isOutput=True)

    with tile.TileContext(nc) as tc, ExitStack() as ctx:
        const = ctx.enter_context(tc.tile_pool(name="const", bufs=1))
        io = ctx.enter_context(tc.tile_pool(name="io", bufs=3))
        mk = ctx.enter_context(tc.tile_pool(name="mk", bufs=3))
        um = ctx.enter_context(tc.tile_pool(name="um", bufs=3))
        sq = ctx.enter_context(tc.tile_pool(name="sq", bufs=3))
        k4p = ctx.enter_context(tc.tile_pool(name="k4p", bufs=3))
        outsb = ctx.enter_context(tc.tile_pool(name="outsb", bufs=4))
        indps = ctx.enter_context(tc.tile_pool(name="indps", bufs=3, space="PSUM"))
        gram = ctx.enter_context(tc.tile_pool(name="gram", bufs=2, space="PSUM"))

        ind_sb = const.tile([128, H], BF16)
        nc.sync.dma_start(out=ind_sb, in_=ind[:])
        ind4_sb = const.tile([128, H], BF16)
        nc.sync.dma_start(out=ind4_sb, in_=ind4[:])

        # PE prewarm: ~4us of junk matmuls so the HAM un-throttles during
        # the pipeline-fill phase instead of during the first real samples.
        warm_ps = ctx.enter_context(
            tc.tile_pool(name="warm", bufs=1, space="PSUM")
        ).tile([H, H], F32)
        for _ in range(60):
            nc.tensor.matmul(warm_ps, ind_sb, ind_sb[:, 0:H], start=True, stop=True)

        mult = mybir.AluOpType.mult

        for s in range(S):
            # ---- loads ----
            abuf = io.tile([128, NB, W], BF16, tag="abuf")
            pn = io.tile([128, NB, 2, W], BF16, tag="pn")
            nc.sync.dma_start(out=abuf, in_=x_a[s].rearrange("(j p) w -> p j w", p=128))
            nc.sync.dma_start(
                out=pn[:, :, 0, :], in_=x_p[s].rearrange("(j p) w -> p j w", p=128)
            )
            nc.sync.dma_start(
                out=pn[:, :, 1, :], in_=x_n[s].rearrange("(j p) w -> p j w", p=128)
            )

            # ---- mask replicas to 128 partitions: one broadcast DMA each ----
            mrep = mk.tile([128, 3, W], BF16, tag="mrep")
            nc.gpsimd.dma_start(out=mrep, in_=masks_rep[s])

            def rep_b(t):
                # [128, NB, W] view of mrep[:, t, :] broadcast over the NB axis
                return mrep[:, t, :].unsqueeze(1).broadcast_to((128, NB, W))

            # ---- masking (DVE): U = ma*a ; vw = [-2*mp*p | -2*mn*n] ----
            ubuf = um.tile([128, NB, W], BF16, tag="ubuf")
            vw = um.tile([128, NB, 2, W + 8], BF16, tag="vw")
            nc.vector.tensor_tensor(out=ubuf, in0=abuf, in1=rep_b(0), op=mult)
            nc.vector.tensor_tensor(
                out=vw[:, :, :, 4 : W + 4],
                in0=pn,
                in1=mrep[:, 1:3, :].unsqueeze(1).broadcast_to((128, NB, 2, W)),
                op=mult,
            )
            # circular wrap columns (both pairs at once)
            nc.vector.tensor_copy(out=vw[:, :, :, 0:4], in_=vw[:, :, :, W : W + 4])
            nc.vector.tensor_copy(out=vw[:, :, :, W + 4 : W + 8], in_=vw[:, :, :, 4:8])

            # ---- squares: u2 on ACT (Square is 1x there; small one goes to
            # ACT, big one to DVE where TT-mul runs 2x) ----
            u2 = sq.tile([128, NB, W], BF16, tag="u2")
            vw2 = sq.tile([128, NB, 2, W], BF16, tag="vw2")
            Sq = mybir.ActivationFunctionType.Square
            nc.scalar.activation(out=u2, in_=ubuf, func=Sq)
            nc.vector.tensor_tensor(
                out=vw2,
                in0=vw[:, :, :, 4 : W + 4],
                in1=vw[:, :, :, 4 : W + 4],
                op=mult,
            )

            # ---- c-reduction via indicator matmul: A/Bp/Bn [32, 512] ----
            a_ps = indps.tile([H, W], F32, tag="ind3")
            b_ps = indps.tile([H, W], F32, tag="ind3")
            c_ps = indps.tile([H, W], F32, tag="ind3")
            for j in range(NB):
                nc.tensor.matmul(a_ps, ind_sb, u2[:, j, :], start=(j == 0), stop=(j == NB - 1))
            for j in range(NB):
                nc.tensor.matmul(b_ps, ind4_sb, vw2[:, j, 0, :], start=(j == 0), stop=(j == NB - 1))
            for j in range(NB):
                nc.tensor.matmul(c_ps, ind4_sb, vw2[:, j, 1, :], start=(j == 0), stop=(j == NB - 1))

            # ---- assemble k4 lhsT [A; ma] and rhs [m2_ext; B2_ext] x pairs ----
            Cp = mybir.ActivationFunctionType.Copy
            k4lhs = k4p.tile([2 * H, W], BF16, tag="k4lhs")
            r44 = k4p.tile([2 * H, 2, W + 8], BF16, tag="r44")
            nc.scalar.activation(out=k4lhs[0:H, :], in_=a_ps, func=Cp)
            nc.gpsimd.dma_start(
                out=k4lhs[H : 2 * H, :], in_=masks[s, 0:H, 4 : W + 4]
            )
            nc.gpsimd.dma_start(
                out=r44[0:H, :, :],
                in_=masks[s, H : 3 * H, :].rearrange("(t p) w -> p t w", p=H),
            )
            nc.scalar.activation(out=r44[H : 2 * H, 0, 4 : W + 4], in_=b_ps, func=Cp)
            nc.scalar.activation(out=r44[H : 2 * H, 1, 4 : W + 4], in_=c_ps, func=Cp)
            nc.vector.tensor_copy(
                out=r44[H : 2 * H, :, 0:4], in_=r44[H : 2 * H, :, W : W + 4]
            )
            nc.vector.tensor_copy(
                out=r44[H : 2 * H, :, W + 4 : W + 8], in_=r44[H : 2 * H, :, 4:8]
            )

            # ---- Gram matmuls (both pairs per matmul via 3D rhs) ----
            num_ps = gram.tile([128, 2, NW], F32, tag="num")
            for j1 in range(JB):
                mb = slice(j1 * 128, (j1 + 1) * 128)
                wn = slice(j1 * 128, j1 * 128 + NW)
                for kc in range(NB):
                    nc.tensor.matmul(
                        num_ps, ubuf[:, kc, mb], vw[:, kc, :, wn],
                        start=(j1 == 0 and kc == 0), stop=False,
                    )
                nc.tensor.matmul(
                    num_ps, k4lhs[:, mb], r44[:, :, wn],
                    start=False, stop=(j1 == JB - 1),
                )
            psb = outsb.tile([128, 2, NW], F32, tag="psb")
            nc.scalar.activation(out=psb, in_=num_ps, func=Cp)
            nc.gpsimd.dma_start(out=raw[s], in_=psb)
    if for_hw:
        _split_waits_pass(nc)
    return nc


def _host_prep(a, p, n, ma, mp, mn):
    bf = ml_dtypes.bfloat16
    A = np.ascontiguousarray(a.reshape(B, R, W)).astype(bf)
    P = np.ascontiguousarray(p.reshape(B, R, W)).astype(bf)
    N = np.ascontiguousarray(n.reshape(B, R, W)).astype(bf)
    M0 = np.concatenate(
        [ma.reshape(B, H, W), mp.reshape(B, H, W), mn.reshape(B, H, W)], axis=1
    ).astype(bf)
    M = np.concatenate([M0[:, :, W - 4 :], M0, M0[:, :, :4]], axis=2)
    # replicas: [b, p, t, w] = mask_t[p % 32, w]; mp/mn rows pre-scaled by -2
    Mr = np.stack(
        [
            np.tile(ma.reshape(B, H, W), (1, 4, 1)),
            np.tile(mp.reshape(B, H, W).astype(np.float32) * -2.0, (1, 4, 1)),
            np.tile(mn.reshape(B, H, W).astype(np.float32) * -2.0, (1, 4, 1)),
        ],
        axis=2,
    ).astype(bf)
    ind = np.zeros((128, H), dtype=bf)
    ind[np.arange(128), np.arange(128) % H] = 1
    ind4 = np.zeros((128, H), dtype=bf)
    ind4[np.arange(128), np.arange(128) % H] = 0.25
    in_maps = []
    for c in range(NCORES):
        sl = slice(c * S, (c + 1) * S)
        in_maps.append(
            {
                "x_a": A[sl],
                "x_p": P[sl],
                "x_n": N[sl],
                "masks": M[sl],
                "masks_rep": Mr[sl],
                "ind": ind,
                "ind4": ind4,
            }
        )
    return in_maps


def _host_den(ma, mp, mn):
    # den counts[b, pair, off] = sum(m1 & roll(m2, off, -1)) over (1,2,3)
    nb = ma.shape[0]
    m1 = ma.reshape(nb, H, W).astype(bool)
    cnts = np.empty((nb, 2, 2 * SHIFT + 1), np.float64)
    for pair, m2 in enumerate((mp, mn)):
        m2 = m2.reshape(nb, H, W).astype(bool)
        for i, off in enumerate(range(-SHIFT, SHIFT + 1)):
            cnts[:, pair, i] = (m1 & np.roll(m2, off, axis=-1)).sum(axis=(1, 2))
    return cnts


def _host_finish(raw_all, cnts):
    # raw_all: [B, 128, 2, NW] float32; g = (num-ap, num-an)
    raw64 = raw_all.astype(np.float64)
    nums = raw64.transpose(0, 2, 1, 3)             # [B, 2, 128, NW]
    idx = np.arange(128)
    dists = []
    for i, off in enumerate(range(-SHIFT, SHIFT + 1)):
        cols = idx + 4 - off
        num = nums[:, :, idx, cols].sum(axis=-1)   # [B, 2]
        dists.append(num / (C * cnts[:, :, i] + 0.001))
    d = np.min(np.stack(dists, axis=0), axis=0)    # [B, 2]
    loss = np.maximum(d[:, 0] - d[:, 1] + MARGIN, 0.0)
    return np.array(loss.mean(), dtype=np.float32)


def kernel(a, p, n, ma, mp, mn):
    global _nc_cache
    from concourse import bass_utils

    if _nc_cache is None:
        _nc_cache = build_nc()
    nc = _nc_cache
    in_maps = _host_prep(a, p, n, ma, mp, mn)
    res = bass_utils.run_bass_kernel_spmd(nc, in_maps, core_ids=list(range(NCORES)))
    raw_all = np.concatenate([res.results[i]["raw"] for i in range(NCORES)], axis=0)
    return _host_finish(raw_all, _host_den(ma, mp, mn))
